# revision 27
# baseline (speedup 1.0000x reference)
"""Trainium2 Bass kernel for a binarized (XNOR-Net) BasicBlock with syncBN.

Computes, for x:[64,128,56,56] f32 and binarized weights:
    out = BN2( qconv(BN1(qconv(x,w1,s2,p1)), w2,s1,p1) + qconv(x,ws,s2,p0) )

Key structure used:
  - sign(x), sign(w) are +-1 -> all three convs are exact in fp8 with fp32
    PSUM accumulation (integer-valued results).
  - The XNOR weight scales alpha=mean|w| fold into the batchnorms: BN is
    scale-invariant except through EPS, which is rescaled on the host.
  - BN1 feeds sign() (g1>0, b1==0 in this problem), so only the per-channel
    mean matters; computed EXACTLY with int32-valued f32 sums.
  - Batch is sharded 8 images/core across 8 NeuronCores; the two BN batch
    stats are synchronized with tiny (1KB) AllReduces.
  - A tiny warmup collective is the very first GpSimd work so the CC
    engine's one-time setup overlaps conv1 instead of blocking BN1.
  - Collective payloads are PE-transposed to a [2,128] row layout so their
    DRAM DMAs are 2 big descriptors instead of a 128-descriptor partition
    gather (which costs ~6us on the critical path).
  - conv1 runs 4 DoubleRow pairs + 1 plain tap per PSUM tile: kh=(0,1)
    pairs via an even/odd row view, the (2,0)/(2,1) pair via an even/odd
    column view.
"""

import os
import sys
from contextlib import ExitStack

import numpy as np

for _p in ("/opt/trn_rl_repo", "/root/.axon_site/_ro/trn_rl_repo"):
    if os.path.isdir(_p) and _p not in sys.path:
        sys.path.insert(0, _p)

import ml_dtypes  # noqa: E402
import concourse.bass as bass  # noqa: E402
import concourse.bacc as bacc  # noqa: E402
import concourse.mybir as mybir  # noqa: E402
import concourse.tile as tile  # noqa: E402
from concourse.bass_utils import run_bass_kernel_spmd  # noqa: E402

F32 = mybir.dt.float32
F16 = mybir.dt.float16
FP8 = mybir.dt.float8e4
NP_FP8 = ml_dtypes.float8_e4m3

N_CORES = 8
NL = 8                      # images per core
CIN = 128
COUT = 256
H = W = 56
OH = OW = 28
PH, PW = 58, 64             # padded conv1 input tile (pad=1, width padded to 64)
P2H, P2W = 30, 32           # padded conv2 input tile (pad=1, width padded to 32)
CHUNK = 392                 # 14 output rows * 28 cols, fits one PSUM bank in f32
NCH = 2                     # chunks per image (2*392 = 784 = 28*28)
ROWS = 14                   # output rows per chunk
COUNT = 64 * OH * OW        # BN reduction count over the full batch (N,H,W)
EPS = 1e-5
ALL_CORES = [list(range(N_CORES))]

LAST_EXEC_NS = None         # set when BASS_TRACE=1
_CACHED_NC = None


def _build_nc():
    nc = bacc.Bacc("TRN2", target_bir_lowering=False, debug=False,
                   num_devices=N_CORES)

    x_in = nc.dram_tensor("xq", [CIN, NL, PH, PW], FP8, kind="ExternalInput")
    w1_in = nc.dram_tensor("w1t", [128, 2, 9, 128], FP8, kind="ExternalInput")
    w2_in = nc.dram_tensor("w2t", [128, 2, 2, 9, 128], FP8, kind="ExternalInput")
    ws_in = nc.dram_tensor("wst", [128, 2, 128], FP8, kind="ExternalInput")
    # aux columns: 0=g1, 1=r (=alphas/alpha2), 2=g2, 3=b2, 4=eps2' (bcast),
    #              5=-g1/COUNT
    aux_in = nc.dram_tensor("aux", [128, 2, 8], F32, kind="ExternalInput")
    id_in = nc.dram_tensor("ident", [128, 128], F32, kind="ExternalInput")
    sel_in = nc.dram_tensor("sel", [16, 2], F32, kind="ExternalInput")
    out_ext = nc.dram_tensor("out", [NL, COUT, OH, OW], F32, kind="ExternalOutput")

    with tile.TileContext(nc) as tc:
        with ExitStack() as ctx:
            _body(ctx, tc, x_in, w1_in, w2_in, ws_in, aux_in, id_in, sel_in,
                  out_ext)

    nc.compile()
    return nc


def _body(ctx, tc, x_in, w1_in, w2_in, ws_in, aux_in, id_in, sel_in, out_ext):
    nc = tc.nc

    const = ctx.enter_context(tc.tile_pool(name="const", bufs=1))
    w1sb = const.tile([128, 2, 9, 128], FP8)     # [ci, cob, tap, co]
    w2sb = const.tile([128, 2, 2, 9, 128], FP8)  # [ciw, cib, cob, tap, co]
    wssb = const.tile([128, 2, 128], FP8)        # [ci, cob, co]
    auxsb = const.tile([128, 2, 8], F32)
    idsb = const.tile([128, 128], F32)
    selsb = const.tile([16, 2], F32)

    big = ctx.enter_context(tc.tile_pool(name="big", bufs=1))
    # c1 (f16, conv1 ints) is dead once xq2 is made; vq (f32) reuses its slot
    c1 = big.tile([128, 2, NL, 784], F16, name="c1", tag="c1vq", bufs=1)
    zs = big.tile([128, 2, NL, 784], F16, name="zs")          # shortcut ints
    xq2 = big.tile([128, 2, NL, P2H, P2W], FP8, name="xq2")   # sign(BN1(conv1))

    stats = ctx.enter_context(tc.tile_pool(name="stats", bufs=1))
    s1strip = stats.tile([128, 2, 16], F32)
    s1tot = stats.tile([128, 2, 1], F32)
    s1row = stats.tile([2, 128], F32)        # transposed S1 rows for DMA out
    s1rg = stats.tile([16, 128], F32)        # gathered S1 rows (8 cores x 2)
    bn1b = stats.tile([128, 2], F32)         # sign bias per cob
    svstrip = stats.tile([128, 2, 2, 16], F32)   # [cob, stat(S/SS), chunk]
    s2t = stats.tile([128, 2, 2, 1], F32)
    s2row = stats.tile([2, 128], F32)        # transposed cob1 stats rows
    s2rg = [stats.tile([16, 128], F32, name=f"s2rg{b}") for b in range(2)]
    bn2 = stats.tile([128, 2, 6], F32)  # 0=negmu,1=musq,2=var,3=rstd,4=scale,5=bias

    dram = ctx.enter_context(tc.tile_pool(name="dram", bufs=1, space="DRAM"))
    cc1_in = dram.tile([2, 128], F32, name="cc1i")
    cc1_out = dram.tile([N_CORES, 2, 128], F32, name="cc1o", addr_space="Shared")
    cc2_in = [dram.tile([2, 128], F32, name=f"cc2i{b}") for b in range(2)]
    cc2_out = [dram.tile([N_CORES, 2, 128], F32, name=f"cc2o{b}",
                         addr_space="Shared") for b in range(2)]

    # ---- warmup collective: very first GpSimd work (1-descriptor input).
    # The CC engine's init is anchored to kernel start (~66us) and the FIRST
    # mesh carries a ~20us extra penalty; this tiny collective absorbs both
    # so BN1's own mesh runs at warm speed (~8us).
    wu_in = dram.tile([8], F32, name="wu_in")
    wu_out = dram.tile([N_CORES, 8], F32, name="wu_out", addr_space="Shared")
    wz = stats.tile([1, 8], F32, name="wz")
    nc.gpsimd.memset(wz[:], 0.0)
    nc.gpsimd.dma_start(wu_in[:], wz[0, :])
    nc.gpsimd.collective_compute(
        "AllGather",
        mybir.AluOpType.bypass,
        replica_groups=ALL_CORES,
        ins=[wu_in[:].opt()],
        outs=[wu_out[:].opt()],
    )

    # zero xq2 padding borders on DVE (interior overwritten by the Sign pass)
    nc.vector.memset(xq2[:, :, :, 0:1, 0:30], 0.0)
    nc.vector.memset(xq2[:, :, :, 29:30, 0:30], 0.0)
    nc.vector.memset(xq2[:, :, :, 1:29, 0:1], 0.0)
    nc.vector.memset(xq2[:, :, :, 1:29, 29:30], 0.0)

    # ---- input DMAs: w1+identity first (4+1 queues), then x images ----
    w1f = w1_in.rearrange("p b t c -> p (b t c)")
    w1sf = w1sb.rearrange("p b t c -> p (b t c)")
    WQ = 2 * 9 * 128 // 4
    for q in range(4):
        nc.sync.dma_start(w1sf[:, q * WQ:(q + 1) * WQ],
                          w1f[:, q * WQ:(q + 1) * WQ])
    nc.sync.dma_start(idsb[:], id_in[:])
    nc.sync.dma_start(selsb[:], sel_in[:])

    xq_pool = ctx.enter_context(tc.tile_pool(name="xqp", bufs=NL))
    xq = [xq_pool.tile([128, PH, PW], FP8, name=f"xq{n}", tag="xq")
          for n in range(NL)]
    xf = x_in.rearrange("p n h w -> p n (h w)")
    # first two images split 4-way for low latency; rest 2-way (1856B lines)
    QCH = PH * PW // 4
    for n in range(2):
        tf = xq[n].rearrange("p h w -> p (h w)")
        for q in range(4):
            nc.sync.dma_start(tf[:, q * QCH:(q + 1) * QCH],
                              xf[:, n, q * QCH:(q + 1) * QCH])
    QCH2 = PH * PW // 2
    for n in range(2, NL):
        tf = xq[n].rearrange("p h w -> p (h w)")
        for q in range(2):
            nc.sync.dma_start(tf[:, q * QCH2:(q + 1) * QCH2],
                              xf[:, n, q * QCH2:(q + 1) * QCH2])
        if n == 3:
            nc.sync.dma_start(w2sb[:], w2_in[:])
            nc.sync.dma_start(wssb[:], ws_in[:])
            nc.sync.dma_start(auxsb[:], aux_in[:])

    psum = ctx.enter_context(tc.tile_pool(name="psum", bufs=7, space="PSUM"))
    ptp = ctx.enter_context(tc.tile_pool(name="ptp", bufs=1, space="PSUM"))
    scr = ctx.enter_context(tc.tile_pool(name="scr", bufs=4))
    ostg_pool = ctx.enter_context(tc.tile_pool(name="ostg", bufs=4))

    # ---------------- conv1: 3x3 stride2 pad1, 128ci -> 256co -------------
    def conv1_rhs(n, kh, kw, ch):
        r0 = kh + 2 * (ROWS * ch)
        return xq[n][:, r0:r0 + 2 * ROWS:2, kw:kw + 2 * OW:2]

    def conv1_rhs_khpair(n, kw, ch):
        # [128, 2(kh 0/1), 14(oh), 28(ow)] for DoubleRow over the kh=(0,1) pair
        v = xq[n].rearrange("p (hp two) w -> p two hp w", two=2)
        return v[:, :, ROWS * ch:ROWS * ch + ROWS, kw:kw + 2 * OW:2]

    def conv1_rhs_kwpair(n, ch):
        # [128, 2(kw 0/1), 14(oh), 28(ow)] for DoubleRow over taps (2,0),(2,1)
        v = xq[n].rearrange("p h (wp two) -> p two h wp", two=2)
        r0 = 2 + 2 * (ROWS * ch)
        return v[:, :, r0:r0 + 2 * ROWS:2, 0:OW]

    for g in range(4):
        for cob in range(2):
            ptiles = [psum.tile([128, CHUNK], F32, tag="ps", name=f"ps{cob}_{g}_{i}")
                      for i in range(4)]
            for ti in range(5):
                if ti < 3:          # DR pair over kh=(0,1), kw=ti
                    lhsT = w1sb[:, cob, ti:ti + 4:3, :]
                elif ti == 3:       # DR pair over taps (2,0),(2,1)
                    lhsT = w1sb[:, cob, 6:8, :]
                else:               # plain tap (2,2)
                    lhsT = w1sb[:, cob, 8, :]
                for li in range(2):
                    n = 2 * g + li
                    for ch in range(NCH):
                        if ti < 3:
                            rhs = conv1_rhs_khpair(n, ti, ch)
                        elif ti == 3:
                            rhs = conv1_rhs_kwpair(n, ch)
                        else:
                            rhs = conv1_rhs(n, 2, 2, ch)
                        nc.tensor.matmul(
                            ptiles[2 * li + ch][:],
                            lhsT,
                            rhs,
                            start=(ti == 0),
                            stop=(ti == 4),
                            perf_mode=(mybir.MatmulPerfMode.DoubleRow
                                       if ti < 4 else None),
                        )
            for li in range(2):
                n = 2 * g + li
                for ch in range(NCH):
                    col = 2 * n + ch
                    _drain(nc, col % 2,
                           c1[:, cob, n, ch * CHUNK:(ch + 1) * CHUNK],
                           ptiles[2 * li + ch][:],
                           s1strip[:, cob, col:col + 1])

    # BN1 AllReduce: reduce -> PE transpose -> 2-descriptor DMA -> trigger
    nc.vector.tensor_reduce(
        out=s1tot[:, :, :], in_=s1strip[:, :, :],
        axis=mybir.AxisListType.X, op=mybir.AluOpType.add,
    )
    pt1 = ptp.tile([128, 128], F32, tag="pt", name="pt1")
    nc.tensor.transpose(pt1[0:2, 0:128], s1tot[:, :, 0], idsb[:])
    nc.vector.tensor_scalar(
        out=s1row[:, :], in0=pt1[0:2, 0:128], scalar1=1.0, scalar2=None,
        op0=mybir.AluOpType.mult)
    nc.gpsimd.dma_start(cc1_in[:], s1row[:, :])
    nc.gpsimd.collective_compute(
        "AllGather",
        mybir.AluOpType.bypass,
        replica_groups=ALL_CORES,
        ins=[cc1_in[:].opt()],
        outs=[cc1_out[:].opt()],
    )

    # ---------------- shortcut: 1x1 stride2 pad0 (overlaps AllReduce) ------
    for cob in range(2):
        for g in range(4):
            ptiles = [psum.tile([128, CHUNK], F32, tag="ps", name=f"pss{cob}_{g}_{i}")
                      for i in range(4)]
            for li in range(2):
                n = 2 * g + li
                for ch in range(NCH):
                    r0 = 1 + 2 * (ROWS * ch)
                    nc.tensor.matmul(
                        ptiles[2 * li + ch][:],
                        wssb[:, cob, :],
                        xq[n][:, r0:r0 + 2 * ROWS:2, 1:1 + 2 * OW:2],
                        start=True, stop=True,
                    )
            for li in range(2):
                n = 2 * g + li
                for ch in range(NCH):
                    _drain(nc, (2 * n + ch + 1) % 2,
                           zs[:, cob, n, ch * CHUNK:(ch + 1) * CHUNK],
                           ptiles[2 * li + ch][:],
                           None)

    # consume BN1: gathered row read (16 lines) -> one PE matmul that both
    # sums over cores and transposes: S1[128ch, 2cob] = gathered^T @ sel
    nc.sync.dma_start(s1rg[:, :], cc1_out.rearrange("r b c -> (r b) c"))
    pt2 = ptp.tile([128, 128], F32, tag="pt", name="pt2")
    nc.tensor.matmul(pt2[0:128, 0:2], s1rg[:, :], selsb[:, :],
                     start=True, stop=True)
    # bias1 = -g1 * S1 / COUNT  (aux col 5 = -g1/COUNT)
    nc.vector.tensor_tensor(
        out=bn1b[:, :], in0=pt2[0:128, 0:2], in1=auxsb[:, :, 5],
        op=mybir.AluOpType.mult)

    # -------- xq2 = Sign(g1*z1 + bias1), one ACT op per (2 images, cob) ----
    c1v = c1.rearrange("p b n (h w) -> p b n h w", h=OH)
    for n in range(0, NL, 2):
        for cob in range(2):
            nc.scalar.activation(
                xq2[:, cob, n:n + 2, 1:1 + 2 * ROWS, 1:1 + OW],
                c1v[:, cob, n:n + 2, :, :],
                mybir.ActivationFunctionType.Sign,
                scale=auxsb[:, cob, 0:1],
                bias=bn1b[:, cob:cob + 1],
            )

    # vq reuses c1's slot (c1 fully consumed by the Sign pass)
    vq = big.tile([128, 2, NL, 784], F32, name="vq", tag="c1vq", bufs=1)

    # ---------------- conv2: 3x3 stride1 pad1, 256ci -> 256co --------------
    TAPS = [(kh, kw) for kh in range(3) for kw in range(3)]
    for cob in range(2):
        for g in range(4):
            ptiles = [psum.tile([128, CHUNK], F32, tag="ps", name=f"ps2{cob}_{g}_{i}")
                      for i in range(4)]
            for t, (kh, kw) in enumerate(TAPS):
                lhsT = w2sb.rearrange(
                    "p cib cob t co -> p cob t cib co")[:, cob, t, :, :]
                for li in range(2):
                    n = 2 * g + li
                    for ch in range(NCH):
                        r0 = kh + ROWS * ch
                        nc.tensor.matmul(
                            ptiles[2 * li + ch][:],
                            lhsT,
                            xq2[:, :, n, r0:r0 + ROWS, kw:kw + OW],
                            start=(t == 0), stop=(t == 8),
                            perf_mode=mybir.MatmulPerfMode.DoubleRow,
                        )
            for li in range(2):
                n = 2 * g + li
                for ch in range(NCH):
                    col = 2 * n + ch
                    sl = slice(ch * CHUNK, (ch + 1) * CHUNK)
                    # vq = r*zs + z2 (one DVE op, channel sums -> S strip)
                    nc.vector.scalar_tensor_tensor(
                        out=vq[:, cob, n, sl],
                        in0=zs[:, cob, n, sl],
                        scalar=auxsb[:, cob, 1:2],
                        in1=ptiles[2 * li + ch][:],
                        op0=mybir.AluOpType.mult,
                        op1=mybir.AluOpType.add,
                        accum_out=svstrip[:, cob, 0, col:col + 1])
                    sqs = scr.tile([128, CHUNK], F32, tag="sqs",
                                   name=f"sq{cob}_{n}_{ch}")
                    nc.scalar.activation(
                        sqs[:], vq[:, cob, n, sl],
                        mybir.ActivationFunctionType.Square,
                        accum_out=svstrip[:, cob, 1, col:col + 1])
        # ---- per-cob BN2 stats AllReduce ----------------------------------
        nc.vector.tensor_reduce(
            out=s2t[:, cob, :, :], in_=svstrip[:, cob, :, :],
            axis=mybir.AxisListType.X, op=mybir.AluOpType.add)
        if cob == 0:
            # overlapped by conv2 cob1: plain (slow, partition-gather) DMA
            # keeps the PE queue free for cob1's matmuls
            nc.gpsimd.dma_start(cc2_in[0].rearrange("s c -> c s"),
                                s2t[:, 0, :, 0])
        else:
            # critical path: PE transpose (PE is idle now) + fast row DMA
            pt3 = ptp.tile([128, 128], F32, tag="pt", name="pt3")
            nc.tensor.transpose(pt3[0:2, 0:128], s2t[:, 1, :, 0], idsb[:])
            nc.vector.tensor_scalar(
                out=s2row[:, :], in0=pt3[0:2, 0:128], scalar1=1.0,
                scalar2=None, op0=mybir.AluOpType.mult)
            nc.gpsimd.dma_start(cc2_in[1][:], s2row[:, :])
        nc.gpsimd.collective_compute(
            "AllGather",
            mybir.AluOpType.bypass,
            replica_groups=ALL_CORES,
            ins=[cc2_in[cob][:].opt()],
            outs=[cc2_out[cob][:].opt()],
        )

    inv_count = 1.0 / COUNT
    for cob in range(2):
        # gathered row read (16 lines) + one PE matmul = core-sum + transpose;
        # cob1's read goes on gpsimd so it isn't queued behind out-DMA issues
        reng = nc.sync if cob == 0 else nc.gpsimd
        reng.dma_start(s2rg[cob][:, :],
                       cc2_out[cob].rearrange("r b c -> (r b) c"))
        ptc = ptp.tile([128, 128], F32, tag="pt", name=f"ptc{cob}")
        nc.tensor.matmul(ptc[0:128, 0:2], s2rg[cob][:, :], selsb[:, :],
                         start=True, stop=True)
        sg = ptc[0:128, 0:2]
        # negmu = -S/COUNT ; musq = negmu^2 ; var = SS/COUNT - musq
        nc.vector.tensor_scalar(
            out=bn2[:, cob, 0:1], in0=sg[:, 0:1],
            scalar1=-inv_count, scalar2=None, op0=mybir.AluOpType.mult)
        nc.vector.tensor_tensor(
            out=bn2[:, cob, 1:2], in0=bn2[:, cob, 0:1], in1=bn2[:, cob, 0:1],
            op=mybir.AluOpType.mult)
        nc.vector.scalar_tensor_tensor(
            out=bn2[:, cob, 2:3], in0=sg[:, 1:2],
            scalar=inv_count, in1=bn2[:, cob, 1:2],
            op0=mybir.AluOpType.mult, op1=mybir.AluOpType.subtract)
        # rstd = 1/sqrt(var + eps2') ; scale = g2*rstd ; bias = negmu*scale + b2
        nc.scalar.activation(
            bn2[:, cob, 3:4], bn2[:, cob, 2:3],
            mybir.ActivationFunctionType.Sqrt,
            bias=auxsb[:, cob, 4:5])
        nc.vector.reciprocal(out=bn2[:, cob, 3:4], in_=bn2[:, cob, 3:4])
        nc.vector.tensor_tensor(
            out=bn2[:, cob, 4:5], in0=bn2[:, cob, 3:4], in1=auxsb[:, cob, 2:3],
            op=mybir.AluOpType.mult)
        nc.vector.scalar_tensor_tensor(
            out=bn2[:, cob, 5:6], in0=bn2[:, cob, 0:1],
            scalar=bn2[:, cob, 4:5], in1=auxsb[:, cob, 3:4],
            op0=mybir.AluOpType.mult, op1=mybir.AluOpType.add)

        # ---- final normalize + store for this cob -------------------------
        # normalize image pairs on different engines (DVE fastest gets odd
        # images), then ONE DMA per pair so the ~700ns-per-issue sequencer
        # time doesn't serialize the tail
        ENG = {0: "act", 1: "dve", 2: "gp", 3: "dve",
               4: "act", 5: "dve", 6: "gp", 7: "dve"}
        for p in range(NL // 2):
            ostg = ostg_pool.tile([128, 2, 784], F32, tag="ostg",
                                  name=f"og{cob}_{p}")
            for h in range(2):
                n = 2 * p + h
                eng = ENG[n]
                if eng == "act":
                    nc.scalar.activation(
                        ostg[:, h, :], vq[:, cob, n, :],
                        mybir.ActivationFunctionType.Identity,
                        scale=bn2[:, cob, 4:5],
                        bias=bn2[:, cob, 5:6])
                elif eng == "dve":
                    nc.vector.tensor_scalar(
                        out=ostg[:, h, :], in0=vq[:, cob, n, :],
                        scalar1=bn2[:, cob, 4:5], scalar2=bn2[:, cob, 5:6],
                        op0=mybir.AluOpType.mult, op1=mybir.AluOpType.add)
                else:
                    nc.gpsimd.tensor_scalar(
                        out=ostg[:, h, :], in0=vq[:, cob, n, :],
                        scalar1=bn2[:, cob, 4:5], scalar2=bn2[:, cob, 5:6],
                        op0=mybir.AluOpType.mult, op1=mybir.AluOpType.add)
            nc.sync.dma_start(
                out_ext[2 * p:2 * p + 2, cob * 128:(cob + 1) * 128, :, :]
                .rearrange("n c h w -> c n (h w)"),
                ostg[:, :, :])


def _drain(nc, use_act, out_ap, psum_ap, strip_ap):
    """PSUM -> SBUF copy + optional per-tile channel sum, on DVE or ACT."""
    if use_act:
        kw = {"accum_out": strip_ap} if strip_ap is not None else {}
        nc.scalar.activation(
            out_ap, psum_ap, mybir.ActivationFunctionType.Copy, **kw)
    elif strip_ap is not None:
        nc.vector.tensor_scalar(
            out=out_ap, in0=psum_ap, scalar1=1.0, scalar2=None,
            op0=mybir.AluOpType.mult, op1=mybir.AluOpType.add,
            accum_out=strip_ap)
    else:
        nc.vector.tensor_scalar(
            out=out_ap, in0=psum_ap, scalar1=1.0, scalar2=None,
            op0=mybir.AluOpType.mult)


def _sign_pm1(a):
    return np.where(a >= 0, np.float32(1.0), np.float32(-1.0))


def _prep_inputs(x, w1, g1, b1, w2, g2, b2, ws):
    """Host-side: binarize + lay out per-core input maps."""
    x = np.asarray(x, np.float32)
    w1 = np.asarray(w1, np.float32)
    w2 = np.asarray(w2, np.float32)
    ws = np.asarray(ws, np.float32)
    g1 = np.asarray(g1, np.float32)
    b1 = np.asarray(b1, np.float32)
    g2 = np.asarray(g2, np.float32)
    b2 = np.asarray(b2, np.float32)

    assert np.all(b1 == 0.0), "kernel's exact BN1-sign path requires b1 == 0"

    alpha2 = np.mean(np.abs(w2), dtype=np.float32)
    alphas = np.mean(np.abs(ws), dtype=np.float32)
    r = np.float32(alphas / alpha2)
    eps2p = np.float32(EPS / (alpha2 * alpha2))

    # weights -> lhsT tap tiles
    w1s = _sign_pm1(w1).reshape(2, 128, 128, 9)          # [cob, co, ci, tap]
    w1t = np.ascontiguousarray(
        w1s.transpose(2, 0, 3, 1)).astype(NP_FP8)        # [ci, cob, tap, co]
    w2s = _sign_pm1(w2).reshape(2, 128, 2, 128, 9)       # [cob, co, cib, ciw, tap]
    w2t = np.ascontiguousarray(
        w2s.transpose(3, 2, 0, 4, 1)).astype(NP_FP8)     # [ciw, cib, cob, tap, co]
    wss = _sign_pm1(ws).reshape(2, 128, 128)             # [cob, co, ci]
    wst = np.ascontiguousarray(wss.transpose(2, 0, 1)).astype(NP_FP8)

    aux = np.zeros((128, 2, 8), np.float32)
    aux[:, :, 0] = g1.reshape(2, 128).T
    aux[:, :, 1] = r
    aux[:, :, 2] = g2.reshape(2, 128).T
    aux[:, :, 3] = b2.reshape(2, 128).T
    aux[:, :, 4] = eps2p
    aux[:, :, 5] = -g1.reshape(2, 128).T / np.float32(COUNT)

    ident = np.eye(128, dtype=np.float32)
    sel = np.zeros((16, 2), np.float32)
    sel[0::2, 0] = 1.0
    sel[1::2, 1] = 1.0

    xs = _sign_pm1(x)  # [64, 128, 56, 56]
    in_maps = []
    for c in range(N_CORES):
        xpad = np.zeros((CIN, NL, PH, PW), np.float32)
        xpad[:, :, 1:57, 1:57] = xs[c * NL:(c + 1) * NL].transpose(1, 0, 2, 3)
        in_maps.append({
            "xq": xpad.astype(NP_FP8),
            "w1t": w1t,
            "w2t": w2t,
            "wst": wst,
            "aux": aux,
            "ident": ident,
            "sel": sel,
        })
    return in_maps


def kernel(x, w1, g1, b1, w2, g2, b2, ws):
    global _CACHED_NC, LAST_EXEC_NS
    if _CACHED_NC is None:
        _CACHED_NC = _build_nc()
    nc = _CACHED_NC

    in_maps = _prep_inputs(x, w1, g1, b1, w2, g2, b2, ws)
    trace = bool(os.environ.get("BASS_TRACE"))
    res = run_bass_kernel_spmd(nc, in_maps, list(range(N_CORES)), trace=trace)
    LAST_EXEC_NS = res.exec_time_ns

    out = np.concatenate([res.results[c]["out"] for c in range(N_CORES)], axis=0)
    return out.astype(np.float32)
